# revision 1
# baseline (speedup 1.0000x reference)
"""Trainium2 Bass kernel: RGB->YUV, median filter (3x3 luma / 9x9 chroma,
symmetric padding), YUV->RGB.  Input image (4,3,512,512) f32.

Strategy (per core; 8 cores take 64-row slabs with 4-row halo):
  - host pre-pads rows (symmetric) and ships a [12, 72, 512] slab per core
  - RGB->YUV elementwise in natural [row-part, x-free] layout
  - PE-transpose to [x-part, row-free]; vertical 9-sorts (shared sorting
    network, one sorted column per pixel) vectorized over (plane, row)
  - PE-transpose sorted columns back to [row-part, x-free]
  - horizontal phase: shared pair-merges P(x)=merge(col x, col x+1) and quad
    merges Q(x)=merge(P(x), P(x+2)) at every x; per pixel, Batcher-merge
    (DCE'd to the rank band [31..40]) of Q(x-4), Q(x) and a selection ladder
    rank40(E72, col(x+4)) = max(E[31], min(E[32],C[8]), ..., min(E[40],C[0]))
  - all min/max on VectorE; transposes on PE; PSUM evacuation + pad copies on
    ScalarE; DMAs on SyncE.
Everything is exact (median selects an element; no approximation).
"""

import numpy as np

# ---------------------------------------------------------------------------
# sorting / selection network generation (pure python, runs at import)
# ---------------------------------------------------------------------------


class _Net:
    def __init__(self):
        self.nodes = []
        self.memo = {}

    def _mk(self, spec):
        if spec in self.memo:
            return self.memo[spec]
        nid = len(self.nodes)
        self.nodes.append(spec)
        self.memo[spec] = nid
        return nid

    def inp(self, key):
        return self._mk(('in', key))

    def vmin(self, a, b):
        if a == b:
            return a
        if a > b:
            a, b = b, a
        return self._mk(('min', a, b))

    def vmax(self, a, b):
        if a == b:
            return a
        if a > b:
            a, b = b, a
        return self._mk(('max', a, b))

    def oe_merge(self, A, B):
        A, B = list(A), list(B)
        if not A:
            return B
        if not B:
            return A
        if len(A) == 1 and len(B) == 1:
            return [self.vmin(A[0], B[0]), self.vmax(A[0], B[0])]
        E = self.oe_merge(A[0::2], B[0::2])
        O = self.oe_merge(A[1::2], B[1::2])
        res = [E[0]]
        i = 0
        while i < len(O) and i + 1 < len(E):
            res += [self.vmin(O[i], E[i + 1]), self.vmax(O[i], E[i + 1])]
            i += 1
        res += O[i:]
        res += E[i + 1:]
        return res

    def oe_sort(self, wires):
        wires = list(wires)
        if len(wires) <= 1:
            return wires
        m = len(wires) // 2
        return self.oe_merge(self.oe_sort(wires[:m]), self.oe_sort(wires[m:]))

    def ancestors(self, out_ids):
        need = set()
        stack = list(out_ids)
        while stack:
            n = stack.pop()
            if n in need:
                continue
            need.add(n)
            spec = self.nodes[n]
            if spec[0] != 'in':
                stack.append(spec[1])
                stack.append(spec[2])
        return need


def _stage_ops(net, out_map, in_map, tag):
    """Linearize the pruned net.  out_map: node->name, in_map: in-key->(name, shift).
    Returns op list [(dst_name, op, (a_name, a_shift), (b_name, b_shift))]."""
    need = sorted(net.ancestors(list(out_map)))
    loc = {}
    ops = []
    for nid in need:
        spec = net.nodes[nid]
        if spec[0] == 'in':
            loc[nid] = in_map[spec[1]]
            continue
        dst = out_map.get(nid, ('tmp', tag, nid))
        ops.append((dst, spec[0], loc[spec[1]], loc[spec[2]]))
        loc[nid] = (dst, 0)
    return ops


def _used_inputs(net, out_ids):
    return sorted({net.nodes[n][1] for n in net.ancestors(out_ids)
                   if net.nodes[n][0] == 'in'})


def _gen_chrom_pass(k=9):
    """Combined op list for one 9x9 phase-H pass over buffers:
    external: ('S', s) s=0..8 (padded sorted columns), ('med',).
    Arena-managed: ('P', t), ('Q', t), ('tmp', ...)."""
    rank = (k * k - 1) // 2  # 40

    pix = _Net()
    QA = [pix.inp(('QA', i)) for i in range(4 * k)]
    QB = [pix.inp(('QB', i)) for i in range(4 * k)]
    C = [pix.inp(('C', i)) for i in range(k)]
    E = pix.oe_merge(QA, QB)
    terms = [E[rank - k]]
    for j in range(k):
        terms.append(pix.vmin(E[rank - j], C[j]))
    med = terms[0]
    for t in terms[1:]:
        med = pix.vmax(med, t)
    need_q = sorted({key[1] for key in _used_inputs(pix, [med])
                     if key[0] in ('QA', 'QB')})

    qnet = _Net()
    qa = [qnet.inp(('PA', i)) for i in range(2 * k)]
    qb = [qnet.inp(('PB', i)) for i in range(2 * k)]
    qout = qnet.oe_merge(qa, qb)
    q_out_map = {qout[t]: ('Q', t) for t in need_q}
    need_p = sorted({key[1] for key in _used_inputs(qnet, list(q_out_map))})

    pnet = _Net()
    pa = [pnet.inp(('SA', i)) for i in range(k)]
    pb = [pnet.inp(('SB', i)) for i in range(k)]
    pout = pnet.oe_merge(pa, pb)
    p_out_map = {pout[t]: ('P', t) for t in need_p}

    ops = []
    # P stage: P[j] = merge(S(x), S(x+1)) at x=j-4; reads S at shifts {0,1}
    ops += [(op[0], op[1], op[2], op[3], 'P') for op in _stage_ops(
        pnet, p_out_map,
        {('SA', i): (('S', i), 0) for i in range(k)} |
        {('SB', i): (('S', i), 1) for i in range(k)}, 'p')]
    # Q stage: Q[j] = merge(P(x), P(x+2)); reads P at shifts {0,2}
    ops += [(op[0], op[1], op[2], op[3], 'Q') for op in _stage_ops(
        qnet, q_out_map,
        {('PA', t): (('P', t), 0) for t in range(2 * k)} |
        {('PB', t): (('P', t), 2) for t in range(2 * k)}, 'q')]
    # pixel stage: reads Q at shifts {0,4}, S at 8
    ops += [(op[0], op[1], op[2], op[3], 'med') for op in _stage_ops(
        pix, {med: ('med', 0)},
        {('QA', t): (('Q', t), 0) for t in range(4 * k)} |
        {('QB', t): (('Q', t), 4) for t in range(4 * k)} |
        {('C', i): (('S', i), 8) for i in range(k)}, 'x')]
    return ops


def _gen_lum_pass(k=3):
    rank = (k * k - 1) // 2  # 4
    pix = _Net()
    P = [pix.inp(('P', i)) for i in range(2 * k)]
    C = [pix.inp(('C', i)) for i in range(k)]
    terms = [P[rank - k]]
    for j in range(k):
        terms.append(pix.vmin(P[rank - j], C[j]))
    med = terms[0]
    for t in terms[1:]:
        med = pix.vmax(med, t)
    need_p = sorted({key[1] for key in _used_inputs(pix, [med])
                     if key[0] == 'P'})

    pnet = _Net()
    pa = [pnet.inp(('SA', i)) for i in range(k)]
    pb = [pnet.inp(('SB', i)) for i in range(k)]
    pout = pnet.oe_merge(pa, pb)
    p_out_map = {pout[t]: ('PL', t) for t in need_p}

    ops = []
    ops += [(op[0], op[1], op[2], op[3], 'PL') for op in _stage_ops(
        pnet, p_out_map,
        {('SA', i): (('SL', i), 0) for i in range(k)} |
        {('SB', i): (('SL', i), 1) for i in range(k)}, 'lp')]
    ops += [(op[0], op[1], op[2], op[3], 'medL') for op in _stage_ops(
        pix, {med: ('medL', 0)},
        {('P', t): (('PL', t), 0) for t in range(2 * k)} |
        {('C', i): (('SL', i), 2) for i in range(k)}, 'lx')]
    return ops


def _gen_colsort(k):
    net = _Net()
    out = net.oe_sort([net.inp(('C', d)) for d in range(k)])
    return _stage_ops(net, {out[s]: ('SV', s) for s in range(k)},
                      {('C', d): (('YV', d), 0) for d in range(k)}, 'cs')


def _arena_assign(ops, external):
    """Assign arena slots to non-external value names.  Returns (slot_of, n)."""
    last_use = {}
    for i, (dst, _op, a, b, _w) in enumerate(ops):
        for src in {a[0], b[0]}:
            if src not in external:
                last_use[src] = i
    slot_of = {}
    free = []
    n = 0
    for i, (dst, _op, a, b, _w) in enumerate(ops):
        if dst not in external and dst not in slot_of:
            if free:
                slot_of[dst] = free.pop()
            else:
                slot_of[dst] = n
                n += 1
        for src in {a[0], b[0]}:
            if src not in external and last_use.get(src) == i:
                free.append(slot_of[src])
    return slot_of, n


_CHROM_OPS = _gen_chrom_pass()
_LUM_OPS = _gen_lum_pass()
_COLSORT9 = _gen_colsort(9)
_COLSORT3 = _gen_colsort(3)

# ---------------------------------------------------------------------------
# bass emission
# ---------------------------------------------------------------------------

RGB2YUV = [[0.299, 0.587, 0.114],
           [-0.14713, -0.28886, 0.436],
           [0.615, -0.51499, -0.10001]]
YUV2RGB = [[1.0, 0.0, 1.13983],
           [1.0, -0.39465, -0.58060],
           [1.0, 2.03211, 0.0]]


def build_nc(H=512, W=512, n_cores=8):
    import concourse.bacc as bacc
    import concourse.bass as bass
    import concourse.mybir as mybir
    import concourse.tile as tile
    from concourse.masks import make_identity

    f32 = mybir.dt.float32
    OUT_ROWS = H // n_cores           # rows per core
    SLAB = OUT_ROWS + 8               # input rows incl. 4+4 halo
    XB = W // 128                     # x blocks
    assert W % 128 == 0 and H % n_cores == 0 and OUT_ROWS >= 1

    PP = 2 * OUT_ROWS                 # partitions used in H layout
    assert PP <= 128
    widths = {'S': W + 8, 'P': W + 6, 'Q': W + 4, 'med': W,
              'SL': W + 2, 'PL': W + 1, 'medL': W}

    chrom_ext = ({('S', s) for s in range(9)} | {('med', 0)})
    chrom_slot, n_chrom_slots = _arena_assign(_CHROM_OPS, chrom_ext)
    lum_ext = ({('SL', s) for s in range(3)} | {('medL', 0)})
    lum_slot, n_lum_slots = _arena_assign(_LUM_OPS, lum_ext)
    csl9_ext = {('SV', s) for s in range(9)} | {('YV', d) for d in range(9)}
    cs9_slot, n_cs9 = _arena_assign(
        [(d, o, a, b, None) for (d, o, a, b) in _COLSORT9], csl9_ext)
    csl3_ext = {('SV', s) for s in range(3)} | {('YV', d) for d in range(3)}
    cs3_slot, n_cs3 = _arena_assign(
        [(d, o, a, b, None) for (d, o, a, b) in _COLSORT3], csl3_ext)

    nc = bacc.Bacc(None, target_bir_lowering=False)
    inp = nc.dram_tensor("inp", [12, SLAB, W], f32, kind="ExternalInput")
    out = nc.dram_tensor("out", [12, OUT_ROWS, W], f32, kind="ExternalOutput")

    AOP = mybir.AluOpType

    def ap3(t, col, npl, pitch, cnt):
        full = t[:]
        off = t[:, col:col + 1].offset
        return bass.AP(full.tensor, off, [list(full.ap[0]), [pitch, npl], [1, cnt]])

    with tile.TileContext(nc) as tc:
        with tc.tile_pool(name="mp", bufs=1) as mp, \
             tc.tile_pool(name="ps", bufs=8, space="PSUM") as psp:

            ident = mp.tile([128, 128], f32, tag="ident", name="ident")
            make_identity(nc, ident[:])

            # persistent per-kernel tiles (reused across halves via WAR)
            YV = [mp.tile([128, 6 * SLAB], f32, tag=f"yv{xb}", name=f"yv{xb}")
                  for xb in range(XB)]
            SV = {(s, par): mp.tile([128, 4 * OUT_ROWS], f32, tag=f"sv{s}_{par}", name=f"sv{s}_{par}")
                  for s in range(9) for par in range(min(2, XB))}
            SVL = {(s, par): mp.tile([128, 2 * OUT_ROWS], f32, tag=f"svl{s}_{par}", name=f"svl{s}_{par}")
                   for s in range(3) for par in range(min(2, XB))}
            HS = {(s, pr): mp.tile([PP, widths['S']], f32, tag=f"hs{s}_{pr}", name=f"hs{s}_{pr}")
                  for s in range(9) for pr in range(2)}
            HSL = [mp.tile([PP, widths['SL']], f32, tag=f"hsl{s}", name=f"hsl{s}")
                   for s in range(3)]
            arena = [mp.tile([PP, widths['S']], f32, tag=f"ar{i}", name=f"ar{i}")
                     for i in range(max(n_chrom_slots, n_lum_slots))]
            med = {pr: mp.tile([PP, W], f32, tag=f"med{pr}", name=f"med{pr}") for pr in range(2)}
            medL = mp.tile([PP, W], f32, tag="medl", name="medl")
            nat = {(b, c): mp.tile([SLAB, W], f32, tag=f"nat{b}_{c}", name=f"nat{b}_{c}")
                   for b in range(2) for c in range(3)}
            # yuvn padded to 128 partitions: PE transpose needs K=128 on HW
            yuvn = {(b, c): mp.tile([128, W], f32, tag=f"yuvn{b}_{c}", name=f"yuvn{b}_{c}")
                    for b in range(2) for c in range(3)}
            for t_ in yuvn.values():
                nc.gpsimd.memset(t_[:], 0.0)
            vs9 = [mp.tile([128, 4 * OUT_ROWS], f32, tag=f"vs9_{i}", name=f"vs9_{i}")
                   for i in range(n_cs9)]
            vs3 = [mp.tile([128, 2 * OUT_ROWS], f32, tag=f"vs3_{i}", name=f"vs3_{i}")
                   for i in range(n_cs3)]
            rgb = [mp.tile([PP, W], f32, tag=f"rgb{c}", name=f"rgb{c}") for c in range(3)]

            def emit_pass(ops, slot_of, sbuf_of, wname_key):
                for dst, opn, (an, ash), (bn, bsh), wkey in ops:
                    wd = widths[wkey if wname_key is None else wname_key]

                    def ref(nm, sh):
                        t = sbuf_of(nm)
                        return t[:, sh:sh + wd]
                    dt_ = sbuf_of(dst)
                    nc.vector.tensor_tensor(
                        dt_[:, 0:wd], ref(an, ash), ref(bn, bsh),
                        AOP.min if opn == 'min' else AOP.max)

            for half in range(2):
                b0 = 2 * half
                # ---- load + color transform (natural layout) ----
                for b in range(2):
                    for c in range(3):
                        p = (b0 + b) * 3 + c
                        nc.sync.dma_start(
                            nat[(b, c)][:],
                            bass.AP(inp, p * SLAB * W, [[W, SLAB], [1, W]]))
                for b in range(2):
                    R, G, B = (nat[(b, 0)], nat[(b, 1)], nat[(b, 2)])
                    for ci in range(3):
                        dst = yuvn[(b, ci)][0:SLAB, :]
                        c0, c1, c2 = RGB2YUV[ci]
                        nc.vector.tensor_scalar(dst, R[:], c0, None, AOP.mult)
                        nc.vector.scalar_tensor_tensor(
                            dst, G[:], c1, dst, AOP.mult, AOP.add)
                        nc.vector.scalar_tensor_tensor(
                            dst, B[:], c2, dst, AOP.mult, AOP.add)
                # ---- transpose YUV into x-partition layout ----
                # plane order in YV free dim: U0,U1,V0,V1,Y0,Y1
                plane_src = [(0, 1), (1, 1), (0, 2), (1, 2), (0, 0), (1, 0)]
                for xb in range(XB):
                    for pl, (b, ci) in enumerate(plane_src):
                        pt = psp.tile([128, 128], f32)
                        nc.tensor.transpose(
                            pt[:], yuvn[(b, ci)][:, xb * 128:(xb + 1) * 128],
                            ident[:])
                        nc.scalar.copy(
                            YV[xb][:, pl * SLAB:(pl + 1) * SLAB],
                            pt[:, 0:SLAB])

                # ---- phase V + transposes to H layout ----
                for xb in range(XB):
                    par = xb % min(2, XB)

                    def sv_of(nm, _s=None):
                        kind = nm[0]
                        if kind == 'SV':
                            return SV[(nm[1], par)]
                        raise KeyError(nm)
                    # chrom column sort: planes 0..3, row offset d
                    for dst, opn, (an, ash), (bn, bsh) in _COLSORT9:
                        def cref(nm):
                            if nm[0] == 'YV':
                                return ap3(YV[xb], nm[1], 4, SLAB, OUT_ROWS)
                            if nm[0] == 'SV':
                                return ap3(SV[(nm[1], par)], 0, 4,
                                           OUT_ROWS, OUT_ROWS)
                            return ap3(vs9[cs9_slot[nm]], 0, 4,
                                       OUT_ROWS, OUT_ROWS)
                        dt_ = (SV[(dst[1], par)] if dst[0] == 'SV'
                               else vs9[cs9_slot[dst]])
                        nc.vector.tensor_tensor(
                            ap3(dt_, 0, 4, OUT_ROWS, OUT_ROWS),
                            cref((an)), cref((bn)),
                            AOP.min if opn == 'min' else AOP.max)
                    # lum column sort: planes 4,5; row offset 3+d
                    for dst, opn, (an, ash), (bn, bsh) in _COLSORT3:
                        def lref(nm):
                            if nm[0] == 'YV':
                                return ap3(YV[xb], 4 * SLAB + 3 + nm[1], 2,
                                           SLAB, OUT_ROWS)
                            if nm[0] == 'SV':
                                return ap3(SVL[(nm[1], par)], 0, 2,
                                           OUT_ROWS, OUT_ROWS)
                            return ap3(vs3[cs3_slot[nm]], 0, 2,
                                       OUT_ROWS, OUT_ROWS)
                        dt_ = (SVL[(dst[1], par)] if dst[0] == 'SV'
                               else vs3[cs3_slot[dst]])
                        nc.vector.tensor_tensor(
                            ap3(dt_, 0, 2, OUT_ROWS, OUT_ROWS),
                            lref((an)), lref((bn)),
                            AOP.min if opn == 'min' else AOP.max)
                    # transposes to H layout
                    for s in range(9):
                        for pr in range(2):
                            pt = psp.tile([128, 128], f32)
                            nc.tensor.transpose(
                                pt[0:PP, :],
                                SV[(s, par)][:, pr * 2 * OUT_ROWS:
                                             (pr + 1) * 2 * OUT_ROWS],
                                ident[:])
                            nc.scalar.copy(
                                HS[(s, pr)][:, 4 + xb * 128:4 + (xb + 1) * 128],
                                pt[0:PP, 0:128])
                    for s in range(3):
                        pt = psp.tile([128, 128], f32)
                        nc.tensor.transpose(pt[0:PP, :], SVL[(s, par)][:],
                                            ident[:])
                        nc.scalar.copy(
                            HSL[s][:, 1 + xb * 128:1 + (xb + 1) * 128],
                            pt[0:PP, 0:128])

                # ---- pads (reflected columns) ----
                for s in range(9):
                    for pr in range(2):
                        t = HS[(s, pr)]
                        full = t[:]
                        nc.scalar.copy(
                            t[:, 0:4],
                            bass.AP(full.tensor, t[:, 7:8].offset,
                                    [list(full.ap[0]), [-1, 4]]))
                        nc.scalar.copy(
                            t[:, W + 4:W + 8],
                            bass.AP(full.tensor, t[:, W + 3:W + 4].offset,
                                    [list(full.ap[0]), [-1, 4]]))
                for s in range(3):
                    t = HSL[s]
                    nc.scalar.copy(t[:, 0:1], t[:, 1:2])
                    nc.scalar.copy(t[:, W + 1:W + 2], t[:, W:W + 1])

                # ---- phase H: chrom passes (pr=0 UU, pr=1 VV) ----
                for pr in range(2):
                    def ext_of(nm):
                        if nm[0] == 'S':
                            return HS[(nm[1], pr)]
                        if nm[0] == 'med':
                            return med[pr]
                        return arena[chrom_slot[nm]]
                    emit_pass(_CHROM_OPS, chrom_slot, ext_of, None)
                # ---- lum pass ----

                def lext_of(nm):
                    if nm[0] == 'SL':
                        return HSL[nm[1]]
                    if nm[0] == 'medL':
                        return medL
                    return arena[lum_slot[nm]]
                emit_pass(_LUM_OPS, lum_slot, lext_of, None)
                nc.vector.tensor_scalar(medL[:], medL[:], 0.0, 1.0,
                                        AOP.max, AOP.min)

                # ---- YUV -> RGB (H layout, partitions = (batch, y)) ----
                U, V, Y = med[0], med[1], medL
                nc.vector.scalar_tensor_tensor(
                    rgb[0][:], V[:], YUV2RGB[0][2], Y[:], AOP.mult, AOP.add)
                nc.vector.scalar_tensor_tensor(
                    rgb[1][:], U[:], YUV2RGB[1][1], Y[:], AOP.mult, AOP.add)
                nc.vector.scalar_tensor_tensor(
                    rgb[1][:], V[:], YUV2RGB[1][2], rgb[1][:], AOP.mult, AOP.add)
                nc.vector.scalar_tensor_tensor(
                    rgb[2][:], U[:], YUV2RGB[2][1], Y[:], AOP.mult, AOP.add)

                for c in range(3):
                    nc.sync.dma_start(
                        bass.AP(out, (b0 * 3 + c) * OUT_ROWS * W,
                                [[3 * OUT_ROWS * W, 2], [W, OUT_ROWS], [1, W]]),
                        rgb[c][:])

    nc.compile()
    return nc


# ---------------------------------------------------------------------------
# host side
# ---------------------------------------------------------------------------

_STATE = {}


def _runner(H=512, W=512, n_cores=8):
    key = (H, W, n_cores)
    if key in _STATE:
        return _STATE[key]
    import jax
    from jax.sharding import Mesh, PartitionSpec
    from jax.experimental.shard_map import shard_map
    import concourse.mybir as mybir
    from concourse.bass2jax import (_bass_exec_p, install_neuronx_cc_hook,
                                    partition_id_tensor)

    nc = build_nc(H, W, n_cores)
    install_neuronx_cc_hook()

    OUT_ROWS = H // n_cores

    # mirror bass2jax.run_bass_via_pjrt's operand packing exactly
    partition_name = (nc.partition_id_tensor.name
                      if nc.partition_id_tensor else None)
    in_names = []
    out_names = []
    out_avals = []
    zero_shapes = []
    for alloc in nc.m.functions[0].allocations:
        if not isinstance(alloc, mybir.MemoryLocationSet):
            continue
        name = alloc.memorylocations[0].name
        if alloc.kind == "ExternalInput":
            if name != partition_name:
                in_names.append(name)
        elif alloc.kind == "ExternalOutput":
            out_names.append(name)
            shape = tuple(alloc.tensor_shape)
            dtype = mybir.dt.np(alloc.dtype)
            out_avals.append(jax.core.ShapedArray(shape, dtype))
            zero_shapes.append((shape, dtype))
    n_params = len(in_names)
    n_outs = len(out_avals)
    in_names.extend(out_names)
    if partition_name is not None:
        in_names.append(partition_name)
    def _body(*args):
        operands = list(args)
        if partition_name is not None:
            operands.append(partition_id_tensor())
        outs = _bass_exec_p.bind(
            *operands,
            out_avals=tuple(out_avals),
            in_names=tuple(in_names),
            out_names=tuple(out_names),
            lowering_input_output_aliases=(),
            sim_require_finite=True,
            sim_require_nnan=True,
            nc=nc,
        )
        return tuple(outs)

    devices = jax.devices()[:n_cores]
    mesh = Mesh(np.asarray(devices), ("core",))
    donate = tuple(range(n_params, n_params + n_outs))
    fn = jax.jit(shard_map(_body, mesh=mesh,
                           in_specs=(PartitionSpec("core"),) * (n_params + n_outs),
                           out_specs=(PartitionSpec("core"),) * n_outs,
                           check_rep=False),
                 donate_argnums=donate, keep_unused=True)

    def run(slabs):
        x = np.concatenate(slabs, axis=0)
        zeros = [np.zeros((n_cores * s[0], *s[1:]), d)
                 for (s, d) in zero_shapes]
        out = fn(x, *zeros)
        return np.asarray(out[0])

    _STATE[key] = (run, nc)
    return _STATE[key]


def kernel(image):
    image = np.asarray(image, dtype=np.float32)
    B, C, H, W = image.shape
    n_cores = 8
    OUT_ROWS = H // n_cores
    run, _ = _runner(H, W, n_cores)
    padded = np.pad(image, ((0, 0), (0, 0), (4, 4), (0, 0)), mode='symmetric')
    slabs = [np.ascontiguousarray(
        padded[:, :, c * OUT_ROWS:c * OUT_ROWS + OUT_ROWS + 8, :]
        .reshape(12, OUT_ROWS + 8, W)) for c in range(n_cores)]
    res = run(slabs)                      # (n_cores*12, OUT_ROWS, W)
    res = res.reshape(n_cores, 4, 3, OUT_ROWS, W)
    return np.concatenate([res[c] for c in range(n_cores)], axis=2)

